# revision 10
# baseline (speedup 1.0000x reference)
"""Trainium2 Bass kernel for a 3-layer GAT + global-mean-pool head (8 NeuronCores).

Strategy (graph-partition / node parallelism):
- 100k nodes dealt degree-balanced into 8 cores x 114 pages x 128 slots.
- Per layer: each core computes xw = h @ W for its slots (bf16 table rows of
  256B), AllGathers the table across cores, then edge phase: dma_gather of
  per-edge source rows (4 int16-indexed banks), attention coefficients
  e = exp(leaky_relu(al_src + ar_dst)) computed on-chip (al via DVE reduce
  of xw*attl, ar via one-hot-transpose matmuls), unnormalized segment softmax
  via one-hot matmul scatter into PSUM (z from an appended e column),
  self-loops folded locally, then normalize + bias + relu per page.
"""
import numpy as np
import ml_dtypes

import concourse.bass as bass
import concourse.bacc as bacc
import concourse.tile as tile
import concourse.mybir as mybir
from concourse.bass_utils import run_bass_kernel_spmd
from concourse.tile_rust import add_dep_helper

bf16 = ml_dtypes.bfloat16
f32 = np.float32

# ---- problem constants (hardcoded per contract) ----
N_NODES = 100000
N_EDGES = 1600000
D = 128
H, C = 4, 32
N_GRAPHS = 64
NEG = 0.2

# ---- sharding geometry ----
P = 128
NCORES = 8
PPC = 114                 # pages per core
SPC = PPC * P             # 14592 slots per core
TOT = SPC * NCORES        # 116736 table rows (slot-indexed)
NBANK = 4
BROWS = TOT // NBANK      # 29184 rows per bank (<= 32767 for int16)
CPB = 4                   # chunks per (page, bank)
NCH = NBANK * CPB         # 16 chunks per page
EPP = NCH * P             # 2048 edge slots per page
GRP = 4                   # pages per dma_gather group
SEG_CAP = CPB * P         # 512 non-self edges per (page, bank)
NODES_PER_PAGE = 110

DBG = dict(n_layers=3, edge=True, head=True, n_groups=None, ar_mm=True)

DT_BF = mybir.dt.bfloat16
DT_F32 = mybir.dt.float32
DT_I16 = mybir.dt.int16


# =====================================================================
# Host preprocessing
# =====================================================================

def _partition_nodes(src, dst):
    """Deal nodes into 912 pages, degree-balanced, then repair (page,bank)
    overflows with within-core moves. Returns per-node (core, page, slot)."""
    npages = NCORES * PPC
    indeg = np.bincount(dst, minlength=N_NODES)
    order = np.argsort(-indeg, kind="stable")
    # snake-deal by global page id; page g -> core g % 8, local page g // 8
    gpage_of_node = np.empty(N_NODES, np.int64)
    pos = 0
    row = 0
    while pos < N_NODES:
        chunk = min(npages, N_NODES - pos)
        idx = np.arange(chunk)
        if row % 2 == 1:
            idx = npages - 1 - idx
        gpage_of_node[order[pos:pos + chunk]] = idx[:chunk]
        pos += chunk
        row += 1
    core_of_node = (gpage_of_node % NCORES).astype(np.int64)
    page_of_node = (gpage_of_node // NCORES).astype(np.int64)

    # per (core,page,bank) segment loads; bank of a node = core//2 region
    # bank(node) = gslot // BROWS = core // 2  (since SPC*2 == BROWS)
    bank_of_node = core_of_node // 2
    # seg[core, page, bank] = sum over nodes in page of in-edges from bank
    src_bank = bank_of_node[src]
    seg = np.zeros((NCORES, PPC, NBANK), np.int64)
    np.add.at(seg, (core_of_node[dst], page_of_node[dst], src_bank), 1)
    # node's per-bank indegree (for repair)
    nb = np.zeros((N_NODES, NBANK), np.int64)
    np.add.at(nb, (dst, src_bank), 1)

    cnt = np.zeros((NCORES, PPC), np.int64)
    np.add.at(cnt, (core_of_node, page_of_node), 1)

    # repair: move nodes between pages of the same core
    nodes_by_cp = {}
    for n in range(N_NODES):
        nodes_by_cp.setdefault((core_of_node[n], page_of_node[n]), []).append(n)
    for c in range(NCORES):
        for _ in range(2000):
            bad = np.argwhere(seg[c] > SEG_CAP)
            if len(bad) == 0:
                break
            pg, b = bad[0]
            cand = nodes_by_cp[(c, pg)]
            n = max(cand, key=lambda x: nb[x, b])
            # move n to the page of this core with most slack in bank b
            slack = SEG_CAP - seg[c, :, b] - nb[n, b]
            slack[cnt[c] >= 126] = -10**9
            pg2 = int(np.argmax(slack))
            assert slack[pg2] >= 0, "repair failed: no page with slack"
            nodes_by_cp[(c, pg)].remove(n)
            nodes_by_cp[(c, pg2)].append(n)
            seg[c, pg] -= nb[n]
            seg[c, pg2] += nb[n]
            cnt[c, pg] -= 1
            cnt[c, pg2] += 1
            page_of_node[n] = pg2
        assert (seg[c] <= SEG_CAP).all(), "bank repair did not converge"
    assert cnt.max() <= P

    slot_of_node = np.empty(N_NODES, np.int64)
    nodes_at = {}
    for (c, pg), lst in nodes_by_cp.items():
        for i, n in enumerate(lst):
            slot_of_node[n] = i
        nodes_at[(c, pg)] = lst
    return core_of_node, page_of_node, slot_of_node, cnt


def preprocess(x, edge_index, batch):
    src = edge_index[0].astype(np.int64)
    dst = edge_index[1].astype(np.int64)
    core_of, page_of, slot_of, cnt = _partition_nodes(src, dst)
    gslot = core_of * SPC + page_of * P + slot_of  # table row id

    # group non-self edges by (dst core, dst page, src bank), then chunkify
    e_core = core_of[dst]
    e_page = page_of[dst]
    e_bank = (core_of[src] // 2)
    key = ((e_core * PPC + e_page) * NBANK + e_bank)
    eorder = np.argsort(key, kind="stable")
    s_sorted = src[eorder]
    d_sorted = dst[eorder]
    k_sorted = key[eorder]
    seg_counts = np.bincount(k_sorted, minlength=NCORES * PPC * NBANK)
    seg_starts = np.concatenate([[0], np.cumsum(seg_counts)])[:-1]

    per_core = []
    for c in range(NCORES):
        gidx_banks = []        # [NBANK][128, PPC*CPB*P/16] int16 wrapped
        # edge slot arrays per page
        dstcol = np.full((PPC, P, NCH), -1.0, np.float32)
        bank_idx_flat = np.zeros((NBANK, PPC * CPB * P), np.int64)
        for pg in range(PPC):
            for b in range(NBANK):
                k = (c * PPC + pg) * NBANK + b
                n = seg_counts[k]
                st = seg_starts[k]
                assert n <= SEG_CAP
                ss = s_sorted[st:st + n]
                dd = d_sorted[st:st + n]
                base = pg * CPB * P
                bank_idx_flat[b, base:base + n] = gslot[ss] - b * BROWS
                # position i within (page,bank) -> chunk c0 = i // P, part = i % P
                ii = np.arange(n)
                kk = b * CPB + ii // P
                pp = ii % P
                dstcol[pg, pp, kk] = slot_of[dd]
        for b in range(NBANK):
            flat = bank_idx_flat[b]                       # [PPC*512]
            w = np.zeros((P, PPC * CPB * P // 16), np.int16)
            i = np.arange(PPC * CPB * P)
            # per call of GRP pages the wrap is call-local, but since each
            # page block is 512 (mult of 16), per-page wrapping == call wrap.
            # wrap per page block of 512:
            blk = 512
            for pg in range(PPC):
                seg = flat[pg * blk:(pg + 1) * blk]
                j = np.arange(blk)
                w[j % 16, pg * (blk // 16) + j // 16] = seg.astype(np.int16)
            for rep in range(1, 8):
                w[16 * rep:16 * (rep + 1), :] = w[:16, :]
            gidx_banks.append(w)

        # per-slot inputs
        my_nodes_mask = core_of == c
        my_slots = (page_of[my_nodes_mask] * P + slot_of[my_nodes_mask])
        my_ids = np.nonzero(my_nodes_mask)[0]
        hT0 = np.zeros((P, SPC), np.float32)
        xs = np.zeros((SPC, D), np.float32)
        xs[my_slots] = x[my_ids]
        hT0 = xs.T.copy()
        pool_oh = np.zeros((SPC, N_GRAPHS), np.float32)
        pool_oh[my_slots, batch[my_ids]] = 1.0
        per_core.append(dict(
            gidx=[g for g in gidx_banks],
            dstcol=dstcol.astype(bf16),
            hT0=hT0.astype(bf16),
            pool_oh=pool_oh.reshape(PPC, P, N_GRAPHS).astype(bf16),
            slots=my_slots, ids=my_ids,
        ))
    return per_core


def prep_consts(inp):
    """Constant tensors shared by all cores."""
    cons = {}
    for i in range(3):
        W = inp[f"W{i+1}"].astype(np.float32)
        al, ar = inp[f"al{i+1}"], inp[f"ar{i+1}"]
        A = np.zeros((D, 2 * H), np.float32)
        attl_pat = np.zeros((D,), np.float32)
        for h in range(H):
            A[h * C:(h + 1) * C, h] = al[h]
            A[h * C:(h + 1) * C, H + h] = ar[h]
            attl_pat[h * C:(h + 1) * C] = al[h]
        cons[f"W{i+1}"] = W.astype(bf16)
        cons[f"WA{i+1}"] = (W @ A).astype(bf16)
        cons[f"attl{i+1}"] = np.tile(attl_pat[None, :], (P, 1)).astype(bf16)
        cons[f"bias{i+1}"] = np.tile(inp[f"b{i+1}"][None, :], (P, 1)).astype(np.float32)
    cons["iota"] = np.tile(np.arange(P, dtype=np.float32)[None, :], (P, 1)).astype(bf16)
    cons["ident"] = np.eye(P, dtype=np.float32).astype(bf16)
    cons["lin1_w"] = inp["lin1_w"].astype(bf16)
    cons["lin2_w"] = inp["lin2_w"].astype(bf16)
    cons["lin1_b"] = np.tile(inp["lin1_b"][None, :], (N_GRAPHS, 1)).astype(np.float32)
    cons["lin2_b"] = np.tile(inp["lin2_b"][None, :], (N_GRAPHS, 1)).astype(np.float32)
    cnt = np.bincount(inp["batch"], minlength=N_GRAPHS).astype(np.float32)
    cons["cnt_inv"] = (1.0 / np.maximum(cnt, 1.0)).reshape(N_GRAPHS, 1).astype(np.float32)
    return cons


# =====================================================================
# Device program
# =====================================================================

def sap(base_ap, off, dims):
    """AP with explicit free dims [(num, stride)...] on top of a tile AP."""
    ap0 = base_ap.ap[0]
    return bass.AP(base_ap.tensor, base_ap.offset + off,
                   [list(ap0)] + [[s, n] for (n, s) in dims])


def build_program():
    nc = bacc.Bacc("TRN2", target_bir_lowering=False, debug=False,
                   num_devices=NCORES)
    AL = mybir.AluOpType
    AF = mybir.ActivationFunctionType

    # ---- I/O ----
    t_hT0 = nc.dram_tensor("hT0", [P, SPC], DT_BF, kind="ExternalInput")
    t_gidx = [nc.dram_tensor(f"gidx{b}", [P, PPC * 32], DT_I16, kind="ExternalInput")
              for b in range(NBANK)]
    t_dstcol = nc.dram_tensor("dstcol", [PPC, P, NCH], DT_BF, kind="ExternalInput")
    t_pool = nc.dram_tensor("pool_oh", [PPC, P, N_GRAPHS], DT_BF, kind="ExternalInput")
    cn = {}
    for nm, shape, dt in [
        ("W1", [D, D], DT_BF), ("W2", [D, D], DT_BF), ("W3", [D, D], DT_BF),
        ("WA1", [D, 2 * H], DT_BF), ("WA2", [D, 2 * H], DT_BF), ("WA3", [D, 2 * H], DT_BF),
        ("attl1", [P, D], DT_BF), ("attl2", [P, D], DT_BF), ("attl3", [P, D], DT_BF),
        ("bias1", [P, D], DT_F32), ("bias2", [P, D], DT_F32), ("bias3", [P, D], DT_F32),
        ("iota", [P, P], DT_BF), ("ident", [P, P], DT_BF),
        ("lin1_w", [D, D], DT_BF), ("lin2_w", [D, 10], DT_BF),
        ("lin1_b", [N_GRAPHS, D], DT_F32), ("lin2_b", [N_GRAPHS, 10], DT_F32),
        ("cnt_inv", [N_GRAPHS, 1], DT_F32),
    ]:
        cn[nm] = nc.dram_tensor(nm, shape, dt, kind="ExternalInput")

    t_outh = nc.dram_tensor("out_h", [SPC, D], DT_F32, kind="ExternalOutput")
    t_outl = nc.dram_tensor("out_logits", [N_GRAPHS, 10], DT_F32, kind="ExternalOutput")

    # ---- internal DRAM ----
    tab_own = [nc.dram_tensor(f"tab_own{l}", [SPC, D], DT_BF) for l in range(3)]
    tab_full = [nc.dram_tensor(f"tab_full{l}", [TOT, D], DT_BF, addr_space="Shared")
                for l in range(3)]
    alar_own = [nc.dram_tensor(f"alar_own{l}", [SPC, 2 * H], DT_BF) for l in range(3)]
    pool_own = nc.dram_tensor("pool_own", [N_GRAPHS, D], DT_F32)
    pool_red = nc.dram_tensor("pool_red", [N_GRAPHS, D], DT_F32, addr_space="Shared")

    groups = []
    pg0 = 0
    while pg0 < PPC:
        npg = min(GRP, PPC - pg0)
        groups.append((pg0, npg))
        pg0 += npg

    with tile.TileContext(nc) as tc:
        with tc.tile_pool(name="const", bufs=1) as cpool, \
             tc.tile_pool(name="hT", bufs=1) as hpool:
            # persistent consts in SBUF
            sb = {}
            for nm in cn:
                shape = list(cn[nm].shape)
                dt = cn[nm].dtype
                t = cpool.tile(shape, dt, tag=f"c_{nm}")
                nc.sync.dma_start(out=t[:, :], in_=cn[nm][:, :])
                sb[nm] = t
            hT_a = hpool.tile([P, SPC], DT_BF, tag="hT0b")
            hT_b = hpool.tile([P, SPC], DT_BF, tag="hT1b")
            hT = [hT_a, hT_b]
            nc.sync.dma_start(out=hT[0][:, :], in_=t_hT0[:, :])

            for L in range(DBG['n_layers']):
                W = sb[f"W{L+1}"]
                WA = sb[f"WA{L+1}"]
                attl = sb[f"attl{L+1}"]
                bias = sb[f"bias{L+1}"]
                hcur = hT[L % 2]
                hnxt = hT[(L + 1) % 2]

                # ---------- phase A: table build ----------
                with tc.tile_pool(name=f"pa{L}", bufs=3) as pa, \
                     tc.tile_pool(name=f"pap{L}", bufs=2, space="PSUM") as pap:
                    for pg in range(PPC):
                        hsl = hcur[:, pg * P:(pg + 1) * P]
                        xw_ps = pap.tile([P, D], DT_F32, tag="xw")
                        nc.tensor.matmul(out=xw_ps[:, :], lhsT=hsl, rhs=W[:, :],
                                         start=True, stop=True)
                        aa_ps = pap.tile([P, 2 * H], DT_F32, tag="aa")
                        nc.tensor.matmul(out=aa_ps[:, :], lhsT=hsl, rhs=WA[:, :],
                                         start=True, stop=True)
                        T_sb = pa.tile([P, D], DT_BF, tag="T")
                        nc.scalar.activation(out=T_sb[:, :], in_=xw_ps[:, :], func=AF.Copy)
                        aa_sb = pa.tile([P, 2 * H], DT_BF, tag="aa_sb")
                        nc.scalar.activation(out=aa_sb[:, :], in_=aa_ps[:, :], func=AF.Copy)
                        nc.sync.dma_start(out=tab_own[L][pg * P:(pg + 1) * P, :],
                                          in_=T_sb[:, :])
                        nc.sync.dma_start(out=alar_own[L][pg * P:(pg + 1) * P, :],
                                          in_=aa_sb[:, :])

                if not DBG['edge']:
                    nc.gpsimd.dma_start(out=t_outh[:, :], in_=tab_own[L][:, :])
                    continue
                # ---------- AllGather ----------
                ag = nc.gpsimd.collective_compute(
                    "AllGather", AL.bypass,
                    replica_groups=[list(range(NCORES))],
                    ins=[tab_own[L].ap().opt()],
                    outs=[tab_full[L].ap().opt()],
                )

                # ---------- phase B: edge phase ----------
                with tc.tile_pool(name=f"pb{L}", bufs=2) as pb, \
                     tc.tile_pool(name=f"pbg{L}", bufs=2) as pbg, \
                     tc.tile_pool(name=f"pbp{L}", bufs=2, space="PSUM") as pbp, \
                     tc.tile_pool(name=f"pbp1{L}", bufs=1, space="PSUM") as pbp1:
                    if L == 2:
                        pool_ps = pbp1.tile([N_GRAPHS, D], DT_F32, tag="poolps")
                    use_groups = groups if DBG['n_groups'] is None else groups[:DBG['n_groups']]
                    for (g0, npg) in use_groups:
                        Gb = []
                        for b in range(NBANK):
                            gi = pbg.tile([P, GRP * 32], DT_I16, tag=f"gi{b}")
                            nc.sync.dma_start(
                                out=gi[:, 0:npg * 32],
                                in_=t_gidx[b][:, g0 * 32:(g0 + npg) * 32])
                            G = pbg.tile([P, GRP * CPB, 256 // 2], DT_BF, tag=f"G{b}")
                            # G free elems per chunk = 128 (bf16); elem_size=128
                            gcall = nc.gpsimd.dma_gather(
                                out_ap=G[:, 0:npg * CPB, :],
                                in_ap=tab_full[L][b * BROWS:(b + 1) * BROWS, :],
                                idxs_ap=gi[:, 0:npg * 32],
                                num_idxs=npg * CPB * P,
                                num_idxs_reg=npg * CPB * P,
                                elem_size=D,
                                single_packet=False,
                            )
                            add_dep_helper(gcall.ins, ag.ins, sync=True,
                                           reason="gather after AG")
                            Gb.append(G)
                        for pgi in range(npg if DBG.get('page_ops', True) else 0):
                            pg = g0 + pgi
                            # loads
                            dc = pb.tile([P, NCH], DT_BF, tag="dc")
                            nc.sync.dma_start(out=dc[:, :], in_=t_dstcol[pg, :, :])
                            aa = pb.tile([P, 2 * H], DT_BF, tag="aa_rb")
                            nc.sync.dma_start(out=aa[:, :],
                                              in_=alar_own[L][pg * P:(pg + 1) * P, :])
                            xo = pb.tile([P, D], DT_BF, tag="xo")
                            nc.sync.dma_start(out=xo[:, :],
                                              in_=tab_own[L][pg * P:(pg + 1) * P, :])

                            # S one-hot [P, NCH, P]
                            S = pb.tile([P, NCH, P], DT_BF, tag="S")
                            nc.vector.tensor_tensor(
                                out=S[:, :, :],
                                in0=sap(dc[:, :], 0, [(NCH, 1), (P, 0)]),
                                in1=sap(sb["iota"][:, :], 0, [(NCH, 0), (P, 1)]),
                                op=AL.is_equal)

                            # ST = transpose(S) per chunk -> psum bf16, one ACT copy
                            ST_ps = pbp1.tile([P, NCH, P], DT_BF, tag="STps")
                            for k in range(NCH):
                                nc.tensor.transpose(out=ST_ps[:, k, :], in_=S[:, k, :],
                                                    identity=sb["ident"][:, :])
                            ST = pb.tile([P, NCH, P], DT_BF, tag="ST")
                            nc.scalar.activation(out=ST[:, :, :], in_=ST_ps[:, :, :],
                                                 func=AF.Copy)

                            # ar expand: 16 matmuls -> ar_ps [P, NCH*H]
                            ar_ps = pbp1.tile([P, NCH * H], DT_F32, tag="arps")
                            for k in range(NCH):
                                nc.tensor.matmul(
                                    out=ar_ps[:, k * H:(k + 1) * H],
                                    lhsT=ST[:, k, :], rhs=aa[:, H:2 * H],
                                    start=True, stop=True)

                            # al: Xa = G * attl ; tree-reduce over c
                            Xa = pb.tile([P, NCH, P], DT_BF, tag="Xa")
                            for b in range(NBANK):
                                nc.vector.tensor_tensor(
                                    out=sap(Xa[:, :, :], b * CPB * P,
                                            [(CPB, P), (H, C), (C, 1)]),
                                    in0=sap(Gb[b][:, :, :], pgi * CPB * 128,
                                            [(CPB, 128), (H, C), (C, 1)]),
                                    in1=sap(attl[:, :], 0, [(CPB, 0), (H, C), (C, 1)]),
                                    op=AL.mult)
                            w = C // 2
                            while w >= 1:
                                nc.vector.tensor_tensor(
                                    out=sap(Xa[:, :, :], 0, [(NCH, P), (H, C), (w, 1)]),
                                    in0=sap(Xa[:, :, :], 0, [(NCH, P), (H, C), (w, 1)]),
                                    in1=sap(Xa[:, :, :], w, [(NCH, P), (H, C), (w, 1)]),
                                    op=AL.add)
                                w //= 2
                            al_e = sap(Xa[:, :, :], 0, [(NCH, P), (H, C)])  # [P,64] strided

                            # logits + leaky + exp
                            t_f = pb.tile([P, NCH * H], DT_F32, tag="tf")
                            nc.vector.tensor_tensor(out=t_f[:, :], in0=al_e,
                                                    in1=ar_ps[:, :], op=AL.add)
                            t2 = pb.tile([P, NCH * H], DT_F32, tag="t2")
                            nc.vector.scalar_tensor_tensor(
                                out=t2[:, :], in0=t_f[:, :], scalar=NEG,
                                op0=AL.mult, op1=AL.max, in1=t_f[:, :])
                            e_bf = pb.tile([P, NCH * H], DT_BF, tag="ebf")
                            nc.scalar.activation(out=e_bf[:, :], in_=t2[:, :], func=AF.Exp)

                            # Gs = [G*e || e] per chunk (132 cols)
                            Gs = pb.tile([P, NCH, 132], DT_BF, tag="Gs")
                            for b in range(NBANK):
                                nc.vector.tensor_tensor(
                                    out=sap(Gs[:, :, :], b * CPB * 132,
                                            [(CPB, 132), (H, C), (C, 1)]),
                                    in0=sap(Gb[b][:, :, :], pgi * CPB * 128,
                                            [(CPB, 128), (H, C), (C, 1)]),
                                    in1=sap(e_bf[:, :], b * CPB * H,
                                            [(CPB, H), (H, 1), (C, 0)]),
                                    op=AL.mult)
                            nc.vector.tensor_copy(
                                out=sap(Gs[:, :, :], 128, [(NCH, 132), (H, 1)]),
                                in_=e_bf[:, :])

                            # scatter matmuls -> page psum [P, 132]
                            pg_ps = pbp.tile([P, 132], DT_F32, tag="pgps")
                            for k in range(NCH):
                                nc.tensor.matmul(
                                    out=pg_ps[:, :], lhsT=S[:, k, :], rhs=Gs[:, k, :],
                                    start=(k == 0), stop=(k == NCH - 1))

                            # self loop fold
                            es = pb.tile([P, H], DT_F32, tag="es")
                            nc.vector.tensor_tensor(out=es[:, :], in0=aa[:, 0:H],
                                                    in1=aa[:, H:2 * H], op=AL.add)
                            nc.vector.scalar_tensor_tensor(
                                out=es[:, :], in0=es[:, :], scalar=NEG,
                                op0=AL.mult, op1=AL.max, in1=es[:, :])
                            es2 = pb.tile([P, H], DT_F32, tag="es2")
                            nc.scalar.activation(out=es2[:, :], in_=es[:, :], func=AF.Exp)
                            O = pb.tile([P, 132], DT_F32, tag="O")
                            ts1 = pb.tile([P, D], DT_F32, tag="ts1")
                            nc.vector.tensor_tensor(
                                out=ts1[:, :],
                                in0=xo[:, :],
                                in1=sap(es2[:, :], 0, [(H, 1), (C, 0)]),
                                op=AL.mult)
                            nc.vector.tensor_tensor(out=O[:, 0:D], in0=pg_ps[:, 0:D],
                                                    in1=ts1[:, :], op=AL.add)
                            nc.vector.tensor_tensor(out=O[:, D:132], in0=pg_ps[:, D:132],
                                                    in1=es2[:, :], op=AL.add)

                            # normalize + bias + relu
                            z2 = pb.tile([P, H], DT_F32, tag="z2")
                            nc.vector.tensor_scalar_max(out=z2[:, :], in0=O[:, D:132],
                                                        scalar1=1e-30)
                            rz = pb.tile([P, H], DT_F32, tag="rz")
                            nc.vector.reciprocal(out=rz[:, :], in_=z2[:, :])
                            hpg = pb.tile([P, D], DT_F32, tag="hpg")
                            nc.vector.tensor_tensor(
                                out=hpg[:, :],
                                in0=O[:, 0:D],
                                in1=sap(rz[:, :], 0, [(H, 1), (C, 0)]),
                                op=AL.mult)
                            nc.vector.tensor_tensor(out=hpg[:, :], in0=hpg[:, :],
                                                    in1=bias[:, :], op=AL.add)
                            hbf = pb.tile([P, D], DT_BF, tag="hbf")
                            nc.scalar.activation(out=hbf[:, :], in_=hpg[:, :], func=AF.Relu)

                            if L == DBG['n_layers'] - 1 and L < 2:
                                hdump = pb.tile([P, D], DT_F32, tag="hdump")
                                nc.scalar.activation(out=hdump[:, :], in_=hpg[:, :],
                                                     func=AF.Relu)
                                nc.sync.dma_start(
                                    out=t_outh[pg * P:(pg + 1) * P, :], in_=hdump[:, :])
                            if L < 2:
                                tr_ps = pbp.tile([P, P], DT_BF, tag="trps")
                                nc.tensor.transpose(out=tr_ps[:, :], in_=hbf[:, :],
                                                    identity=sb["ident"][:, :])
                                nc.scalar.activation(out=hnxt[:, pg * P:(pg + 1) * P],
                                                     in_=tr_ps[:, :], func=AF.Copy)
                            else:
                                poh = pb.tile([P, N_GRAPHS], DT_BF, tag="poh")
                                nc.sync.dma_start(out=poh[:, :], in_=t_pool[pg, :, :])
                                nc.tensor.matmul(
                                    out=pool_ps[:, :], lhsT=poh[:, :], rhs=hbf[:, :],
                                    start=(pg == 0), stop=(pg == PPC - 1))
                                hrelu = pb.tile([P, D], DT_F32, tag="hrelu")
                                nc.scalar.activation(out=hrelu[:, :], in_=hpg[:, :],
                                                     func=AF.Relu)
                                nc.sync.dma_start(
                                    out=t_outh[pg * P:(pg + 1) * P, :], in_=hrelu[:, :])

                    if L == 2:
                        psb = hpool.tile([N_GRAPHS, D], DT_F32, tag="psb")
                        nc.vector.tensor_copy(out=psb[:, :], in_=pool_ps[:, :])
                        nc.sync.dma_start(out=pool_own[:, :], in_=psb[:, :])

                if L == 2 and DBG['head']:
                    # ---- head (phase-B pools closed) ----
                    with tc.tile_pool(name="hd", bufs=1) as hd, \
                         tc.tile_pool(name="hdp", bufs=1, space="PSUM") as hdp:
                        cc2 = nc.gpsimd.collective_compute(
                            "AllReduce", AL.add,
                            replica_groups=[list(range(NCORES))],
                            ins=[pool_own.ap().opt()],
                            outs=[pool_red.ap().opt()],
                        )
                        pr = hd.tile([N_GRAPHS, D], DT_F32, tag="pr")
                        d0 = nc.sync.dma_start(out=pr[:, :], in_=pool_red[:, :])
                        add_dep_helper(d0.ins, cc2.ins, sync=True,
                                       reason="pool after allreduce")
                        pooled = hd.tile([P, D], DT_BF, tag="pooled")
                        nc.vector.memset(pooled[:, :], 0.0)
                        nc.vector.tensor_scalar(
                            out=pooled[0:N_GRAPHS, :], in0=pr[:, :],
                            scalar1=sb["cnt_inv"][:, :], scalar2=None,
                            op0=AL.mult)
                        ptr_ps = hdp.tile([P, P], DT_BF, tag="ptr")
                        nc.tensor.transpose(out=ptr_ps[:, :], in_=pooled[:, :],
                                            identity=sb["ident"][:, :])
                        pooledT = hd.tile([P, P], DT_BF, tag="pooledT")
                        nc.scalar.activation(out=pooledT[:, :], in_=ptr_ps[:, :],
                                             func=AF.Copy)
                        g_ps = hdp.tile([N_GRAPHS, D], DT_F32, tag="gps")
                        nc.tensor.matmul(out=g_ps[:, :],
                                         lhsT=pooledT[:, 0:N_GRAPHS],
                                         rhs=sb["lin1_w"][:, :],
                                         start=True, stop=True)
                        gsb = hd.tile([N_GRAPHS, D], DT_F32, tag="gsb")
                        nc.vector.tensor_tensor(out=gsb[:, :], in0=g_ps[:, :],
                                                in1=sb["lin1_b"][:, :], op=AL.add)
                        gbf = hd.tile([P, D], DT_BF, tag="gbf")
                        nc.vector.memset(gbf[:, :], 0.0)
                        nc.scalar.activation(out=gbf[0:N_GRAPHS, :], in_=gsb[:, :],
                                             func=AF.Relu)
                        gtr_ps = hdp.tile([P, P], DT_BF, tag="gtr")
                        nc.tensor.transpose(out=gtr_ps[:, :], in_=gbf[:, :],
                                            identity=sb["ident"][:, :])
                        gT = hd.tile([P, P], DT_BF, tag="gT")
                        nc.scalar.activation(out=gT[:, :], in_=gtr_ps[:, :],
                                             func=AF.Copy)
                        l_ps = hdp.tile([N_GRAPHS, 10], DT_F32, tag="lps")
                        nc.tensor.matmul(out=l_ps[:, :], lhsT=gT[:, 0:N_GRAPHS],
                                         rhs=sb["lin2_w"][:, :],
                                         start=True, stop=True)
                        lg = hd.tile([N_GRAPHS, 10], DT_F32, tag="lg")
                        nc.vector.tensor_tensor(out=lg[:, :], in0=l_ps[:, :],
                                                in1=sb["lin2_b"][:, :], op=AL.add)
                        mx = hd.tile([N_GRAPHS, 1], DT_F32, tag="mx")
                        nc.vector.tensor_reduce(out=mx[:, :], in_=lg[:, :],
                                                axis=mybir.AxisListType.X, op=AL.max)
                        tsub = hd.tile([N_GRAPHS, 10], DT_F32, tag="tsub")
                        nc.vector.tensor_scalar(out=tsub[:, :], in0=lg[:, :],
                                                scalar1=mx[:, :], scalar2=None,
                                                op0=AL.subtract)
                        ex = hd.tile([N_GRAPHS, 10], DT_F32, tag="ex")
                        nc.scalar.activation(out=ex[:, :], in_=tsub[:, :], func=AF.Exp)
                        sm = hd.tile([N_GRAPHS, 1], DT_F32, tag="sm")
                        nc.vector.tensor_reduce(out=sm[:, :], in_=ex[:, :],
                                                axis=mybir.AxisListType.X, op=AL.add)
                        lsm = hd.tile([N_GRAPHS, 1], DT_F32, tag="lsm")
                        nc.scalar.activation(out=lsm[:, :], in_=sm[:, :], func=AF.Ln)
                        outl = hd.tile([N_GRAPHS, 10], DT_F32, tag="outl")
                        nc.vector.tensor_scalar(out=outl[:, :], in0=tsub[:, :],
                                                scalar1=lsm[:, :], scalar2=None,
                                                op0=AL.subtract)
                        nc.sync.dma_start(out=t_outl[:, :], in_=outl[:, :])
    nc.compile()
    return nc


# =====================================================================
# Entry point
# =====================================================================

TIMING = {}
TRACE = False

def kernel(**inputs):
    x = np.asarray(inputs["x"], np.float32)
    edge_index = np.asarray(inputs["edge_index"], np.int32)
    batch = np.asarray(inputs["batch"], np.int32)

    per_core = preprocess(x, edge_index, batch)
    cons = prep_consts(inputs)

    nc = build_program()

    in_maps = []
    for c in range(NCORES):
        pc = per_core[c]
        m = dict(
            hT0=np.ascontiguousarray(pc["hT0"]),
            dstcol=np.ascontiguousarray(pc["dstcol"]),
            pool_oh=np.ascontiguousarray(pc["pool_oh"]),
        )
        for b in range(NBANK):
            m[f"gidx{b}"] = np.ascontiguousarray(pc["gidx"][b])
        m.update({k: np.ascontiguousarray(v) for k, v in cons.items()})
        in_maps.append(m)

    import time as _time
    _t0 = _time.time()
    res = run_bass_kernel_spmd(nc, in_maps, core_ids=list(range(NCORES)),
                               trace=TRACE)
    TIMING['run_wall_s'] = _time.time() - _t0
    TIMING['exec_time_ns'] = res.exec_time_ns

    last = np.zeros((N_NODES, D), np.float32)
    for c in range(NCORES):
        oh = res.results[c]["out_h"]
        pc = per_core[c]
        last[pc["ids"]] = oh[pc["slots"]]
    logits = res.results[0]["out_logits"].astype(np.float32)
    return logits, last
